# revision 1
# baseline (speedup 1.0000x reference)
"""Adaptive weighted knowledge-distillation loss on 8 TRN2 NeuronCores.

Data-parallel: batch rows sharded 8 ways. Each core computes, per row r of
its [512, 32000] shard of student logits s and teacher logits t:

    E1 = sum_c e^{t}        F1 = sum_c t * e^{t}
    E4 = sum_c e^{t/4}      A4 = sum_c t * e^{t/4}
    H1 = sum_c e^{s}        B4 = sum_c s * e^{t/4}
    H4 = sum_c e^{s/4}      picked = s[target]

from which (T=4, no max-shift needed: logits are N(0,1), far inside the f32
range of exp, and the loss is exactly shift-invariant):

    entropy = log E1 - F1/E1          alpha = clip(1 - entropy/ln C, 0, 1)
    ce      = log H1 - picked
    kl      = (A4 - B4)/(4*E4) - log E4 + log H4
    loss_r  = (1-alpha)*ce + alpha*T^2*kl

Each core reduces its 512 per-row losses on-device (PE ones-matmul over
partitions) and outputs (local sum)/B; the host sums the 8 per-core
scalars (gather/unshard). Set KD_COLLECTIVE=1 for an on-device AllReduce
instead (+~40 us: a 4-byte AllReduce pays the ~20-30 us collective
latency floor at the very end of the kernel where nothing can hide it).

Engine budget per core (the memory-regime target is the ~364 us HBM read
of the two 65.5 MB logit shards; measured ~490 us, ACT-bound):
  ACT: 4 exp passes @ (N+224)/1.2GHz with accum_out (E1,E4,H1,H4 free)
  DVE: 3 fused scalar_tensor_tensor passes @ (N+58)/0.96GHz (F1, A4, B4)
       (tensor_tensor_reduce crashes this runtime; STT+accum_out is the
        production-safe equivalent, and all accum-bearing DVE ops run 1x
        regardless of dtype, so bf16 buys nothing here)
  PE/gpsimd: final 512-row partition reduction + indirect-DMA target
       gather; sync-engine DMAs stream the chunks.
"""

import os

import numpy as np

import concourse.bacc as bacc
import concourse.bass as bass
import concourse.tile as tile
from concourse import mybir
from concourse.bass_utils import run_bass_kernel_spmd

B, C = 4096, 32000
NCORES = 8
R = B // NCORES      # rows per core
P = 128              # SBUF partitions
NT = R // P          # row tiles per core
CH = 5344            # max column chunk (uneven chunks, sum = C)
_CHUNK_LIST = [5344, 5344, 5328, 5328, 5328, 5328]
_CHUNK_LIST0 = [2672, 2672, 5344, 5328, 5328, 5328, 5328]
_CHUNK_LIST3 = [5344, 5344, 5328, 5328, 5328, 2664, 2664]
assert sum(_CHUNK_LIST) == sum(_CHUNK_LIST0) == sum(_CHUNK_LIST3) == C
NCH = 7              # max chunks per row tile


def _chunks(it):
    # halved first chunk on the first tile (engines start sooner); halved
    # last chunk on the last tile (shorter DVE tail after the final exp)
    if it == 0:
        return _CHUNK_LIST0
    if it == NT - 1:
        return _CHUNK_LIST3
    return _CHUNK_LIST
T = 4.0
LN_C = float(np.log(np.float32(C), dtype=np.float32))

FP32 = mybir.dt.float32
BF16 = mybir.dt.bfloat16
I32 = mybir.dt.int32
ALU = mybir.AluOpType
ACTF = mybir.ActivationFunctionType
AX = mybir.AxisListType


def _build_body(tc, t_dram, s_dram, tgt_dram, out_dram):
    nc = tc.nc

    with (
        tc.tile_pool(name="tin", bufs=2) as tin_pool,
        tc.tile_pool(name="sin", bufs=2) as sin_pool,
        tc.tile_pool(name="wex", bufs=2) as wex_pool,
        tc.tile_pool(name="jnk", bufs=4) as jnk_pool,
        tc.tile_pool(name="acc", bufs=1) as acc_pool,
        tc.tile_pool(name="cac", bufs=3) as cac_pool,
        tc.tile_pool(name="fin", bufs=1) as fin_pool,
        tc.tile_pool(name="ps", bufs=1, space="PSUM") as psum_pool,
        tc.tile_pool(name="dram", bufs=1, space="DRAM") as dram_pool,
    ):
        qnames = ["E1", "E4", "H1", "H4", "F1", "A4", "B4"]
        q = {
            n: acc_pool.tile([P, NT], FP32, name=f"acc_{n}", tag=f"acc_{n}")
            for n in qnames
        }
        picked = acc_pool.tile([P, NT], FP32, tag="acc_picked")

        # --- target gather: flat offsets r*C + tgt[r], r = it*P + p ---
        tgt_sb = acc_pool.tile([P, NT], I32, tag="acc_tgt")
        nc.sync.dma_start(
            out=tgt_sb[:],
            in_=tgt_dram[:].rearrange("(t p) one -> p (t one)", p=P),
        )
        rowbase = acc_pool.tile([P, NT], I32, tag="acc_rowbase")
        nc.gpsimd.iota(
            rowbase[:], pattern=[[P, NT]], base=0, channel_multiplier=1
        )
        flatoff = acc_pool.tile([P, NT], I32, tag="acc_flatoff")
        nc.vector.tensor_scalar_mul(out=flatoff[:], in0=rowbase[:], scalar1=C)
        nc.vector.tensor_tensor(
            out=flatoff[:], in0=flatoff[:], in1=tgt_sb[:], op=ALU.add
        )
        s_flat = s_dram[:].rearrange("r c -> (r c)")[:, None]
        if os.environ.get("KD_NO_GATHER"):
            nc.vector.memset(picked[:], 0.0)
        else:
          for it in range(NT):
            nc.gpsimd.indirect_dma_start(
                out=picked[:, it : it + 1],
                out_offset=None,
                in_=s_flat,
                in_offset=bass.IndirectOffsetOnAxis(
                    ap=flatoff[:, it : it + 1], axis=0
                ),
            )

        # Chunks whose teacher e^t is computed on DVE as (e^{t/4})^2^2
        # instead of on ACT — balances the two engines (ACT is otherwise
        # the bottleneck at 4 exp passes vs DVE's 3 product passes).
        n_dve_w1 = int(os.environ.get("KD_DVE_W1_CHUNKS", "0"))
        total_chunks = NT * NCH
        dve_w1_set = {
            (g * total_chunks) // n_dve_w1 + (total_chunks // (2 * n_dve_w1))
            for g in range(n_dve_w1)
        }

        # --- main streaming loop ---
        for it in range(NT):
            r0 = it * P
            ncc = len(_chunks(it))
            cacc = {
                n: cac_pool.tile([P, ncc], FP32, name=f"cac_{n}", tag=f"cac_{n}")
                for n in qnames
            }
            jacc = (
                cac_pool.tile([P, ncc], FP32, name="jacc", tag="jacc")
                if n_dve_w1
                else None
            )
            c0 = 0
            for ic, ch in enumerate(_chunks(it)):
                tt = tin_pool.tile([P, CH], FP32, name="tt")[:, :ch]
                ss = sin_pool.tile([P, CH], FP32, name="ss")[:, :ch]
                nc.sync.dma_start(out=tt[:], in_=t_dram[r0 : r0 + P, c0 : c0 + ch])
                nc.sync.dma_start(out=ss[:], in_=s_dram[r0 : r0 + P, c0 : c0 + ch])
                c0 += ch

                w1 = wex_pool.tile([P, CH], FP32, name="w1", tag="w1")[:, :ch]
                w4 = wex_pool.tile([P, CH], FP32, name="w4", tag="w4")[:, :ch]
                nc.scalar.activation(
                    out=w4[:], in_=tt[:], func=ACTF.Exp, scale=1.0 / T,
                    accum_out=cacc["E4"][:, ic : ic + 1],
                )
                if it * NCH + ic in dve_w1_set:
                    # DVE path: w1 = (w4^2)^2, E1 rides the second square
                    w2 = wex_pool.tile([P, CH], FP32, name="w2", tag="w2")[:, :ch]
                    nc.vector.scalar_tensor_tensor(
                        out=w2[:], in0=w4[:], scalar=0.0, in1=w4[:],
                        op0=ALU.bypass, op1=ALU.mult,
                        accum_out=jacc[:, ic : ic + 1],
                    )
                    nc.vector.scalar_tensor_tensor(
                        out=w1[:], in0=w2[:], scalar=0.0, in1=w2[:],
                        op0=ALU.bypass, op1=ALU.mult,
                        accum_out=cacc["E1"][:, ic : ic + 1],
                    )
                else:
                    nc.scalar.activation(
                        out=w1[:], in_=tt[:], func=ACTF.Exp,
                        accum_out=cacc["E1"][:, ic : ic + 1],
                    )
                j1 = jnk_pool.tile([P, 1], FP32, tag="jact")
                nc.scalar.activation(
                    out=j1[:].broadcast_to((P, ch)), in_=ss[:], func=ACTF.Exp,
                    accum_out=cacc["H1"][:, ic : ic + 1],
                )
                j2 = jnk_pool.tile([P, 1], FP32, tag="jact")
                nc.scalar.activation(
                    out=j2[:].broadcast_to((P, ch)), in_=ss[:], func=ACTF.Exp,
                    scale=1.0 / T,
                    accum_out=cacc["H4"][:, ic : ic + 1],
                )
                # DVE: fused multiply-reduce (TENSOR_SCALAR_PTR with accum)
                for name, a, b in (("F1", tt, w1), ("A4", tt, w4), ("B4", ss, w4)):
                    jd = jnk_pool.tile([P, 1], FP32, name=f"jd_{name}", tag="jdve")
                    nc.vector.scalar_tensor_tensor(
                        out=jd[:].broadcast_to((P, ch)), in0=a[:], scalar=0.0,
                        in1=b[:],
                        op0=ALU.bypass, op1=ALU.mult,
                        accum_out=cacc[name][:, ic : ic + 1],
                    )
            for n in qnames:
                nc.vector.reduce_sum(
                    out=q[n][:, it : it + 1], in_=cacc[n][:], axis=AX.X
                )

        # --- per-row finalize on [P, NT] ---
        def ftile(name):
            return fin_pool.tile([P, NT], FP32, name=f"fin_{name}", tag=f"fin_{name}")

        rE1, rE4 = ftile("rE1"), ftile("rE4")
        nc.vector.reciprocal(out=rE1[:], in_=q["E1"][:])
        nc.vector.reciprocal(out=rE4[:], in_=q["E4"][:])
        logs = {}
        for n in ("E1", "E4", "H1", "H4"):
            logs[n] = ftile(f"log{n}")
            nc.scalar.activation(out=logs[n][:], in_=q[n][:], func=ACTF.Ln)

        ent = ftile("ent")
        nc.vector.tensor_tensor(out=ent[:], in0=q["F1"][:], in1=rE1[:], op=ALU.mult)
        nc.vector.tensor_tensor(
            out=ent[:], in0=logs["E1"][:], in1=ent[:], op=ALU.subtract
        )
        alpha = ftile("alpha")
        # alpha = 1 - ent/lnC, clipped to [0, 1]
        nc.vector.tensor_scalar(
            out=alpha[:], in0=ent[:],
            scalar1=-1.0 / LN_C, scalar2=1.0,
            op0=ALU.mult, op1=ALU.add,
        )
        nc.vector.tensor_scalar_max(out=alpha[:], in0=alpha[:], scalar1=0.0)
        nc.vector.tensor_scalar_min(out=alpha[:], in0=alpha[:], scalar1=1.0)

        ce = ftile("ce")
        nc.vector.tensor_tensor(
            out=ce[:], in0=logs["H1"][:], in1=picked[:], op=ALU.subtract
        )

        kl = ftile("kl")
        nc.vector.tensor_tensor(
            out=kl[:], in0=q["A4"][:], in1=q["B4"][:], op=ALU.subtract
        )
        nc.vector.tensor_tensor(out=kl[:], in0=kl[:], in1=rE4[:], op=ALU.mult)
        nc.vector.tensor_scalar_mul(out=kl[:], in0=kl[:], scalar1=1.0 / T)
        nc.vector.tensor_tensor(
            out=kl[:], in0=kl[:], in1=logs["E4"][:], op=ALU.subtract
        )
        nc.vector.tensor_tensor(out=kl[:], in0=kl[:], in1=logs["H4"][:], op=ALU.add)

        # loss = ce + alpha*(T^2*kl - ce)
        loss = ftile("loss")
        nc.vector.tensor_scalar_mul(out=loss[:], in0=kl[:], scalar1=T * T)
        nc.vector.tensor_tensor(out=loss[:], in0=loss[:], in1=ce[:], op=ALU.subtract)
        nc.vector.tensor_tensor(out=loss[:], in0=loss[:], in1=alpha[:], op=ALU.mult)
        nc.vector.tensor_tensor(out=loss[:], in0=loss[:], in1=ce[:], op=ALU.add)

        rowsum = fin_pool.tile([P, 1], FP32, tag="fin_rowsum")
        nc.vector.reduce_sum(out=rowsum[:], in_=loss[:], axis=AX.X)

        ones = fin_pool.tile([P, 1], FP32, tag="fin_ones")
        nc.vector.memset(ones[:], 1.0 / B)
        part_ps = psum_pool.tile([1, 1], FP32)
        nc.tensor.matmul(
            out=part_ps[:], lhsT=rowsum[:], rhs=ones[:], start=True, stop=True
        )
        part_sb = fin_pool.tile([1, 1], FP32, tag="fin_part")
        nc.vector.tensor_copy(out=part_sb[:], in_=part_ps[:])

        if not os.environ.get("KD_COLLECTIVE"):
            nc.sync.dma_start(out=out_dram[:], in_=part_sb[:])
            return
        part_dram = dram_pool.tile([1, 1], FP32, tag="dram_part")
        total_dram = dram_pool.tile([1, 1], FP32, tag="dram_total")
        nc.gpsimd.dma_start(out=part_dram[:], in_=part_sb[:])
        nc.gpsimd.collective_compute(
            "AllReduce",
            ALU.add,
            replica_groups=[list(range(NCORES))],
            ins=[part_dram[:].opt()],
            outs=[total_dram[:].opt()],
        )
        nc.gpsimd.dma_start(out=out_dram[:], in_=total_dram[:])


_CACHED_NC = None


def _build():
    global _CACHED_NC
    if _CACHED_NC is not None:
        return _CACHED_NC
    nc = bacc.Bacc(
        "TRN2", target_bir_lowering=False, debug=False, num_devices=NCORES
    )
    t_dram = nc.dram_tensor("teacher", [R, C], FP32, kind="ExternalInput")
    s_dram = nc.dram_tensor("student", [R, C], FP32, kind="ExternalInput")
    tgt_dram = nc.dram_tensor("targets_i32", [R, 1], I32, kind="ExternalInput")
    out_dram = nc.dram_tensor("out", [1, 1], FP32, kind="ExternalOutput")
    with tile.TileContext(nc) as tc:
        _build_body(tc, t_dram, s_dram, tgt_dram, out_dram[:])
    nc.compile()
    _CACHED_NC = nc
    return nc


def kernel(outputs, teacher_outputs, targets, _results_out=None):
    outputs = np.ascontiguousarray(np.asarray(outputs, dtype=np.float32))
    teacher_outputs = np.ascontiguousarray(
        np.asarray(teacher_outputs, dtype=np.float32)
    )
    targets = np.asarray(targets)
    assert outputs.shape == (B, C) and teacher_outputs.shape == (B, C)
    tgt32 = targets.astype(np.int32).reshape(B, 1)

    nc = _build()
    in_maps = []
    for i in range(NCORES):
        sl = slice(i * R, (i + 1) * R)
        in_maps.append(
            {
                "teacher": teacher_outputs[sl],
                "student": outputs[sl],
                "targets_i32": np.ascontiguousarray(tgt32[sl]),
            }
        )
    res = run_bass_kernel_spmd(nc, in_maps, core_ids=list(range(NCORES)))
    if _results_out is not None:
        _results_out.append(res)
    if os.environ.get("KD_COLLECTIVE"):
        return np.float32(res.results[0]["out"].reshape(()))
    # gather/unshard: each core returns its (local loss sum)/B partial
    return np.float32(sum(np.float32(r["out"].reshape(())) for r in res.results))

